# revision 59
# baseline (speedup 1.0000x reference)
"""Trainium2 Bass kernel for a single DeBERTa-style attention head.

Problem shapes (hardcoded):
  B=8, S=2048, E=768(n_embed), H=64(head)
  q = I @ Wq + bq ; k = x @ Wk + bk ; v = x @ Wv + bv
  w = (q @ k^T) / sqrt(E) ; w = where(mask==0, -1e9, w)
  scores = softmax(w, axis=-1) ; out = scores @ v

Sharding: data-parallel over batch B across the 8 NeuronCores (one batch
element per core, identical SPMD program).

v2 changes vs v1 (v1 measured 104us, bottleneck = serialized fp32 input
DMA with a ~30us dead start before any compute):
  - I^T and x^T are cast to bf16 on the host, halving their HBM bytes
    (12.6MB -> 6.3MB per core) and making them plain copies eligible for
    HWDGE on the sync queue.  The mask uint8->bf16 cast-DMAs stay on the
    gpsimd SWDGE queue, which now streams CONCURRENTLY from t=0 instead
    of queueing behind 12.6MB of fp32 input.
  - Input DMAs are chunked (I^T by 128-row E-chunks, x^T by 512-column
    S-blocks) with the q/k/v projection matmuls ordered to trail the
    stream chunk-by-chunk, so the PE starts ~2us in (also warming the
    PE p-state ramp) instead of waiting for a whole 6.3MB tile.
  - v-projection PSUM->SBUF copies moved to gpsimd to unload DVE.

Per-core dataflow (bf16 operands, fp32 PSUM accumulation):
  1. qT [64,S] = Wq^T I^T accumulated per E-chunk as chunks land; kT
     per 512-col S-block; v per 128-col k-chunk + bias + a ones column
     (v_aug) so the softmax denominator falls out of the ctx matmul's
     extra output column.
  2. k-chunk-major attention: w^T-chunk [128k, q] = kT-chunk^T @ qT;
     e = exp(w^T / sqrt(E)) on ACT straight from PSUM (no row max
     needed: |w/sqrt(E)| is O(1) so exp cannot overflow, and softmax is
     shift-invariant); s^T = e * mask^T (multiplicative masking matches
     the reference's -1e9 additive mask, which underflows to exactly 0
     after softmax); ctx[q-chunk, 0:65] accumulates s^T-chunk^T @ v_aug
     over all 16 k-chunks in PSUM.
  3. out = ctx[:,0:64] * (1/ctx[:,64]).
"""

import math
from contextlib import ExitStack

import numpy as np

import concourse.bass as bass
import concourse.tile as tile
import concourse.mybir as mybir
from concourse import bacc
from concourse.bass_utils import run_bass_kernel_spmd

B, S, E, H = 8, 2048, 768, 64
N_CORES = 8
SC = S // 128   # 16 seq chunks
EC = E // 128   # 6 embed chunks
NB = 4          # 512-col S-blocks
SCALE = 1.0 / math.sqrt(E)

F32 = mybir.dt.float32
BF16 = mybir.dt.bfloat16
U8 = mybir.dt.uint8
FP8 = mybir.dt.float8e4
AF = mybir.ActivationFunctionType
ALU = mybir.AluOpType

_cache = {}


def _build_program():
    nc = bacc.Bacc("TRN2", target_bir_lowering=False, debug=False)

    # All bulk inputs are host-packed partition-major so every DMA
    # descriptor is a >=4KB contiguous run (per-queue throughput is
    # descriptor-count-bound: ~7ns/packet at the queue head).
    dIT = nc.dram_tensor("IT", [E, S], BF16, kind="ExternalInput")
    dXT = nc.dram_tensor("XTp", [128, NB, EC, 512], BF16, kind="ExternalInput")
    dmT = nc.dram_tensor("maskTp", [128, SC, S], U8, kind="ExternalInput")
    dW = nc.dram_tensor("Wpack", [128, EC, 3 * H], BF16, kind="ExternalInput")
    dB = nc.dram_tensor("bpack", [H, 2 * H], F32, kind="ExternalInput")
    dbv = nc.dram_tensor("bv", [1, H], BF16, kind="ExternalInput")
    dout = nc.dram_tensor("out", [S, H], F32, kind="ExternalOutput")

    with tile.TileContext(nc) as tc, ExitStack() as ctx:
        singles = ctx.enter_context(tc.tile_pool(name="singles", bufs=1))

        # --- weights first (tiny, unblock the first matmuls) ---
        # DMA issue costs the issuing engine ~1.2us each, so spread the
        # streams: sync issues W + x^T blocks, scalar issues I^T chunks +
        # biases on its own hardware queue (concurrent), gpsimd the mask.
        w_all = singles.tile([128, EC, 3 * H], BF16, tag="Wpack")
        nc.sync.dma_start(out=w_all, in_=dW.ap())
        b_all = singles.tile([H, 2 * H], F32, tag="bpack")
        bv_t = singles.tile([1, H], BF16, tag="bv")
        w_sb = {
            "Wq": w_all[:, :, 0:H],
            "Wk": w_all[:, :, H:2 * H],
            "Wv": w_all[:, :, 2 * H:3 * H],
        }
        b_sb = {"bq": b_all[:, 0:1], "bk": b_all[:, H:H + 1], "bv": bv_t}

        ones_row = singles.tile([1, 128], BF16, tag="ones")
        nc.vector.memset(ones_row, 1.0)
        # hoist the ACT exp-table load off the critical path, before the
        # scalar engine's DMA issues
        act_warm = singles.tile([1, 2], BF16, tag="act_warm")
        nc.scalar.activation(act_warm, ones_row[0:1, 0:2], AF.Exp, scale=SCALE)

        # --- bulk input streams ---
        # sync/HWDGE queue: I^T in 6 E-chunks (contiguous 128 rows x 4KB),
        # then x^T in 4 512-col S-blocks (768 rows x 1KB runs).
        # x^T block 0 first so k/v projection work starts (and warms the
        # PE p-state with real work) while I^T streams in behind it.
        IT = singles.tile([128, EC, S], BF16, tag="IT")
        XT = singles.tile([128, EC, S], BF16, tag="XT")

        def dma_xt(blk):
            nc.sync.dma_start(
                out=XT[:, :, blk * 512:(blk + 1) * 512],
                in_=dXT.ap()[:, blk],
            )

        # biases ride the otherwise-idle scalar queue; everything bulky
        # stays on sync: x^T block 0 first (k0/v0 warm the PE with real
        # work from ~10us), then the full I^T stream (gates qT, which
        # gates ALL scores), then x^T blocks 1-3.
        nc.scalar.dma_start(out=b_all, in_=dB.ap())
        nc.scalar.dma_start(out=bv_t, in_=dbv.ap())
        dma_xt(0)
        for ei in range(EC):
            nc.sync.dma_start(
                out=IT[:, ei, :], in_=dIT.ap()[ei * 128:(ei + 1) * 128, :]
            )
        for blk in range(1, NB):
            dma_xt(blk)

        # gpsimd/SWDGE queue (concurrent with the above): whole mask^T as
        # uint8 -> bf16 cast-DMAs in 8 duo-chunks.  The DMA engines are
        # write-bus-bound (~360GB/s aggregate) and the expanded mask is
        # 8MB of SBUF writes vs 6.6MB for all inputs, so duos 2..7 are
        # gated on x^T block arrival (tiny gpsimd reads) to keep the mask
        # stream from starving the inputs that unblock the score phase.
        maskT_all = singles.tile([128, SC, S], BF16, tag="maskT")
        for g in range(SC // 2):
            # Real dependency (not just engine order, which the tile
            # scheduler may reorder): copy one input element INTO this
            # duo's destination region, so the cast-DMA write-after-writes
            # the gate and cannot start before that input chunk landed.
            # Keeps the 8MB of mask SBUF writes from starving the input
            # stream on the shared (write-bound) DMA engines.
            if g == 0:
                gate_src = None          # first duo flows immediately
            elif g == 1:
                gate_src = IT[0:1, 3, 0:1]
            elif g == 2:
                gate_src = IT[0:1, EC - 1, 0:1]
            else:
                # later duos wait for x^T blocks: the mask streams almost
                # entirely UNDER the score phase, not before it
                blk = min(g - 2, NB - 1)
                gate_src = XT[0:1, 0, blk * 512:blk * 512 + 1]
            if gate_src is not None:
                nc.gpsimd.tensor_copy(maskT_all[0:1, 2 * g, 0:1], gate_src)
            nc.gpsimd.dma_start(
                out=maskT_all[:, 2 * g:2 * g + 2, :],
                in_=dmT.ap()[:, 2 * g:2 * g + 2, :],
            )

        qT = singles.tile([64, S], BF16, tag="qT")
        kT = singles.tile([64, S], BF16, tag="kT")
        vA = singles.tile([128, SC, 66], BF16, tag="vA")
        # ones column for the softmax-denominator trick, set once
        nc.vector.memset(vA[:, :, H:H + 1], 1.0)

        sp = ctx.enter_context(tc.tile_pool(name="sp", bufs=9))
        eep = ctx.enter_context(tc.tile_pool(name="eep", bufs=4))
        outp = ctx.enter_context(tc.tile_pool(name="outp", bufs=1))

        def emit_score(ki, psw):
            """w^T-chunk -> exp -> mask multiply; returns the sT tile.

            Both w halves are emitted before both exps (and both exps
            before both multiplies) so each engine sees its two ops
            back-to-back."""
            maskT_sb = maskT_all[:, ki, :]
            sT_sb = sp.tile([128, S], BF16, tag="sT")
            wps = []
            for hh in range(2):
                wp = psw.tile([128, 1024], F32, tag="w")
                for nb in range(2):
                    nc.tensor.matmul(
                        wp[:, nb * 512:(nb + 1) * 512],
                        lhsT=kT[:, ki * 128:(ki + 1) * 128],
                        rhs=qT[:, (hh * 2 + nb) * 512:(hh * 2 + nb + 1) * 512],
                        start=True,
                        stop=True,
                    )
                wps.append(wp)
            e_sbs = []
            for hh in range(2):
                e_sb = eep.tile([128, 1024], BF16, tag="e")
                nc.scalar.activation(e_sb, wps[hh], AF.Exp, scale=SCALE)
                e_sbs.append(e_sb)
            for hh in range(2):
                nc.vector.tensor_tensor(
                    sT_sb[:, hh * 1024:(hh + 1) * 1024],
                    e_sbs[hh],
                    maskT_sb[:, hh * 1024:(hh + 1) * 1024],
                    ALU.mult,
                )
            return sT_sb

        sTs = {}
        # q projection: E-chunk-major, trailing the I^T stream.  Each nb
        # slice of the accumulator is one 2KB bank so accumulation groups
        # stay bank-local.
        with tc.tile_pool(name="psA", bufs=2, space="PSUM") as psA:

            def emit_k_block(blk):
                psk = psA.tile([64, 512], F32, tag="pk")
                for ei in range(EC):
                    nc.tensor.matmul(
                        psk,
                        lhsT=w_sb["Wk"][:, ei, :],
                        rhs=XT[:, ei, blk * 512:(blk + 1) * 512],
                        start=(ei == 0),
                        stop=(ei == EC - 1),
                    )
                nc.vector.tensor_scalar(
                    kT[:, blk * 512:(blk + 1) * 512], psk, b_sb["bk"], None,
                    ALU.add,
                )

            def emit_v_proj(kb):
                psv = psA.tile([128, H], F32, tag="pv")
                for ei in range(EC):
                    nc.tensor.matmul(
                        psv,
                        lhsT=XT[:, ei, kb * 128:(kb + 1) * 128],
                        rhs=w_sb["Wv"][:, ei, :],
                        start=(ei == 0),
                        stop=False,
                    )
                nc.tensor.matmul(
                    psv,
                    lhsT=ones_row[:, 0:128],
                    rhs=b_sb["bv"],
                    start=False,
                    stop=True,
                )
                nc.vector.tensor_copy(vA[:, kb, 0:H], psv)

            # k0/v0 first: x^T block 0 heads the sync queue, so this is
            # real PE work from ~10us that also warms the p-state while
            # I^T streams in behind it.
            with tc.tile_pool(name="psQ", bufs=1, space="PSUM") as psQ:
                emit_k_block(0)
                for kb in range(4):
                    emit_v_proj(kb)
                # q projection: E-chunk-major, trailing the I^T stream
                psqall = psQ.tile([64, NB, 512], F32, tag="pq")
                for ei in range(EC):
                    for nb in range(NB):
                        nc.tensor.matmul(
                            psqall[:, nb, :],
                            lhsT=w_sb["Wq"][:, ei, :],
                            rhs=IT[:, ei, nb * 512:(nb + 1) * 512],
                            start=(ei == 0),
                            stop=(ei == EC - 1),
                        )
                # bias-adds split DVE / ACT so qT's two halves complete in
                # parallel (S(0)'s first w-matmul needs only nb 0-1)
                for nb in (0, 1):
                    nc.vector.tensor_scalar(
                        qT[:, nb * 512:(nb + 1) * 512], psqall[:, nb, :],
                        b_sb["bq"], None, ALU.add,
                    )
                for nb in (2, 3):
                    nc.scalar.activation(
                        qT[:, nb * 512:(nb + 1) * 512], psqall[:, nb, :],
                        AF.Identity, bias=b_sb["bq"],
                    )

            # Scores stream from here, ACT-bound at ~2.26us/chunk.  All
            # remaining k-blocks and v-projections are spread between
            # score chunks in arrival order: each <=1.3us PE insertion
            # fits under ACT's one-chunk (2.26us) pipeline buffer, while
            # any bigger lump would stall the exp stream.  S(0-3) run
            # BEFORE k1: x^T block 1 lands mid-S(0-3), so k1 never stalls
            # the in-order PE.
            with tc.tile_pool(name="psw1", bufs=2, space="PSUM") as psw1:
                sTs[0] = emit_score(0, psw1)
                sTs[1] = emit_score(1, psw1)
                emit_k_block(1)
                sTs[2] = emit_score(2, psw1)
                emit_v_proj(4)
                emit_v_proj(5)
                sTs[3] = emit_score(3, psw1)
                emit_k_block(2)
                sTs[4] = emit_score(4, psw1)
                emit_v_proj(6)
                emit_v_proj(7)
                sTs[5] = emit_score(5, psw1)
                emit_v_proj(8)
                emit_v_proj(9)
                sTs[6] = emit_score(6, psw1)
                emit_k_block(3)
                emit_v_proj(10)
                emit_v_proj(11)
                sTs[7] = emit_score(7, psw1)
                for kb in range(12, 16):
                    emit_v_proj(kb)

        # ---- ctx accumulation (psA's + psw1's banks freed above) ----
        psctx = ctx.enter_context(
            tc.tile_pool(name="psctx", bufs=1, space="PSUM")
        )
        psw2 = ctx.enter_context(
            tc.tile_pool(name="psw2", bufs=2, space="PSUM")
        )

        # [q_within, qj, 64 ctx + 1 denom + pad] — 128-wide regions keep
        # each accumulation group inside one PSUM bank.
        ctxall = psctx.tile([128, SC, 128], F32, tag="ctxall")

        def emit_ctx(ki):
            # start=True zeroes the whole 2KB PSUM bank, so only the first
            # matmul touching each bank (4 qj regions per bank) gets it;
            # stop on the bank's last matmul.
            sT_sb = sTs.pop(ki)
            for qj in range(SC):
                nc.tensor.matmul(
                    ctxall[:, qj, 0:H + 1],
                    lhsT=sT_sb[:, qj * 128:(qj + 1) * 128],
                    rhs=vA[:, ki, 0:H + 1],
                    start=(ki == 0 and qj % 4 == 0),
                    stop=(ki == SC - 1 and qj % 4 == 3),
                )

        for ki in range(8, SC):
            sTs[ki] = emit_score(ki, psw2)
            emit_ctx(ki - 8)
        for ki in range(8, SC):
            emit_ctx(ki)

        # epilogue split by PSUM bank (4 qj per bank): each bank's
        # reciprocal/multiply/output-DMA starts as soon as its last ctx
        # accumulation closes, instead of waiting for the whole ctx tile
        recip_t = outp.tile([128, SC, 1], F32, tag="recip")
        o_all = outp.tile([128, SC, H], F32, tag="o")
        dview = dout.ap().rearrange("(p qj) h -> p qj h", p=128)
        for bank in range(4):
            qlo, qhi = 4 * bank, 4 * bank + 4
            nc.vector.reciprocal(
                recip_t[:, qlo:qhi, :], ctxall[:, qlo:qhi, H:H + 1]
            )
            rsl = recip_t[:, qlo:qhi, :]
            recip_bcast = bass.AP(
                tensor=rsl.tensor,
                offset=rsl.offset,
                ap=[rsl.ap[0], rsl.ap[1], [0, H]],
            )
            nc.vector.tensor_tensor(
                o_all[:, qlo:qhi, :], ctxall[:, qlo:qhi, 0:H], recip_bcast,
                ALU.mult,
            )
            nc.sync.dma_start(
                out=dview[:, qlo:qhi, :], in_=o_all[:, qlo:qhi, :]
            )

    nc.compile()
    return nc


def get_program():
    if "nc" not in _cache:
        _cache["nc"] = _build_program()
    return _cache["nc"]


def make_in_maps(I, x, mask, Wq, bq, Wk, bk, Wv, bv):
    import ml_dtypes

    BF = ml_dtypes.bfloat16
    FP8NP = ml_dtypes.float8_e4m3
    I = np.asarray(I, dtype=np.float32)
    x = np.asarray(x, dtype=np.float32)
    mask = np.asarray(mask, dtype=np.int32)

    Wpack = np.concatenate(
        [
            np.asarray(Wq, dtype=np.float32).astype(BF),
            np.asarray(Wk, dtype=np.float32).astype(BF),
            np.asarray(Wv, dtype=np.float32).astype(BF),
        ],
        axis=1,
    )
    # partition-major: Wp[p, ec, h] = Wpack[ec*128 + p, h]
    Wp = np.ascontiguousarray(
        Wpack.reshape(EC, 128, 3 * H).transpose(1, 0, 2)
    )
    bpack = np.concatenate(
        [
            np.broadcast_to(np.asarray(bq, np.float32).reshape(H, 1), (H, H)),
            np.broadcast_to(np.asarray(bk, np.float32).reshape(H, 1), (H, H)),
        ],
        axis=1,
    ).astype(np.float32)
    bv = np.asarray(bv, dtype=np.float32).reshape(1, H).astype(BF)

    in_maps = []
    for b in range(B):
        xT = x[b].T.astype(BF)                     # [E, S]
        # XTp[p, blk, ec, c] = xT[ec*128 + p, blk*512 + c]
        XTp = np.ascontiguousarray(
            xT.reshape(EC, 128, NB, 512).transpose(1, 2, 0, 3)
        )
        mT = mask[b].T.astype(np.uint8)            # [S(k), S(q)]
        # maskTp[p, ki, q] = mT[ki*128 + p, q]
        mTp = np.ascontiguousarray(
            mT.reshape(SC, 128, S).transpose(1, 0, 2)
        )
        in_maps.append(
            {
                "IT": np.ascontiguousarray(I[b].T).astype(BF),
                "XTp": XTp,
                "maskTp": mTp,
                "Wpack": Wp, "bpack": bpack, "bv": bv,
            }
        )
    return in_maps


def unpermute_out(dev_out):
    # device wrote row (p*16 + qj) = logical q row (qj*128 + p)
    return (
        np.asarray(dev_out).reshape(128, SC, H).transpose(1, 0, 2).reshape(S, H)
    )


def kernel(I, x, mask, Wq, bq, Wk, bk, Wv, bv):
    nc = get_program()
    in_maps = make_in_maps(I, x, mask, Wq, bq, Wk, bk, Wv, bv)
    res = run_bass_kernel_spmd(nc, in_maps, list(range(N_CORES)))
    out = np.stack(
        [unpermute_out(res.results[b]["out"]) for b in range(B)], axis=0
    )
    return out.astype(np.float32)


# revision 61
# speedup vs baseline: 1.1325x; 1.1325x over previous
"""Trainium2 Bass kernel for a single DeBERTa-style attention head.

Problem shapes (hardcoded):
  B=8, S=2048, E=768(n_embed), H=64(head)
  q = I @ Wq + bq ; k = x @ Wk + bk ; v = x @ Wv + bv
  w = (q @ k^T) / sqrt(E) ; w = where(mask==0, -1e9, w)
  scores = softmax(w, axis=-1) ; out = scores @ v

Sharding: data-parallel over batch B across the 8 NeuronCores (one batch
element per core, identical SPMD program).

v2 changes vs v1 (v1 measured 104us, bottleneck = serialized fp32 input
DMA with a ~30us dead start before any compute):
  - I^T and x^T are cast to bf16 on the host, halving their HBM bytes
    (12.6MB -> 6.3MB per core) and making them plain copies eligible for
    HWDGE on the sync queue.  The mask uint8->bf16 cast-DMAs stay on the
    gpsimd SWDGE queue, which now streams CONCURRENTLY from t=0 instead
    of queueing behind 12.6MB of fp32 input.
  - Input DMAs are chunked (I^T by 128-row E-chunks, x^T by 512-column
    S-blocks) with the q/k/v projection matmuls ordered to trail the
    stream chunk-by-chunk, so the PE starts ~2us in (also warming the
    PE p-state ramp) instead of waiting for a whole 6.3MB tile.
  - v-projection PSUM->SBUF copies moved to gpsimd to unload DVE.

Per-core dataflow (bf16 operands, fp32 PSUM accumulation):
  1. qT [64,S] = Wq^T I^T accumulated per E-chunk as chunks land; kT
     per 512-col S-block; v per 128-col k-chunk + bias + a ones column
     (v_aug) so the softmax denominator falls out of the ctx matmul's
     extra output column.
  2. k-chunk-major attention: w^T-chunk [128k, q] = kT-chunk^T @ qT;
     e = exp(w^T / sqrt(E)) on ACT straight from PSUM (no row max
     needed: |w/sqrt(E)| is O(1) so exp cannot overflow, and softmax is
     shift-invariant); s^T = e * mask^T (multiplicative masking matches
     the reference's -1e9 additive mask, which underflows to exactly 0
     after softmax); ctx[q-chunk, 0:65] accumulates s^T-chunk^T @ v_aug
     over all 16 k-chunks in PSUM.
  3. out = ctx[:,0:64] * (1/ctx[:,64]).
"""

import math
from contextlib import ExitStack

import numpy as np

import concourse.bass as bass
import concourse.tile as tile
import concourse.mybir as mybir
from concourse import bacc
from concourse.bass_utils import run_bass_kernel_spmd

B, S, E, H = 8, 2048, 768, 64
N_CORES = 8
SC = S // 128   # 16 seq chunks
EC = E // 128   # 6 embed chunks
NB = 4          # 512-col S-blocks
SCALE = 1.0 / math.sqrt(E)

F32 = mybir.dt.float32
BF16 = mybir.dt.bfloat16
U8 = mybir.dt.uint8
FP8 = mybir.dt.float8e4
AF = mybir.ActivationFunctionType
ALU = mybir.AluOpType

_cache = {}


def _build_program():
    nc = bacc.Bacc("TRN2", target_bir_lowering=False, debug=False)

    # All bulk inputs are host-packed partition-major so every DMA
    # descriptor is a >=4KB contiguous run (per-queue throughput is
    # descriptor-count-bound: ~7ns/packet at the queue head).
    dIT = nc.dram_tensor("IT", [E, S], BF16, kind="ExternalInput")
    dXT = nc.dram_tensor("XTp", [128, NB, EC, 512], BF16, kind="ExternalInput")
    dmT = nc.dram_tensor("maskTp", [128, SC, S], U8, kind="ExternalInput")
    dW = nc.dram_tensor("Wpack", [128, EC, 3 * H], BF16, kind="ExternalInput")
    dB = nc.dram_tensor("bpack", [H, 2 * H], F32, kind="ExternalInput")
    dbv = nc.dram_tensor("bv", [1, H], BF16, kind="ExternalInput")
    dout = nc.dram_tensor("out", [S, H], F32, kind="ExternalOutput")

    with tile.TileContext(nc) as tc, ExitStack() as ctx:
        singles = ctx.enter_context(tc.tile_pool(name="singles", bufs=1))

        # --- weights first (tiny, unblock the first matmuls) ---
        # DMA issue costs the issuing engine ~1.2us each, so spread the
        # streams: sync issues W + x^T blocks, scalar issues I^T chunks +
        # biases on its own hardware queue (concurrent), gpsimd the mask.
        w_all = singles.tile([128, EC, 3 * H], BF16, tag="Wpack")
        nc.sync.dma_start(out=w_all, in_=dW.ap())
        b_all = singles.tile([H, 2 * H], F32, tag="bpack")
        bv_t = singles.tile([1, H], BF16, tag="bv")
        w_sb = {
            "Wq": w_all[:, :, 0:H],
            "Wk": w_all[:, :, H:2 * H],
            "Wv": w_all[:, :, 2 * H:3 * H],
        }
        b_sb = {"bq": b_all[:, 0:1], "bk": b_all[:, H:H + 1], "bv": bv_t}

        ones_row = singles.tile([1, 128], BF16, tag="ones")
        nc.vector.memset(ones_row, 1.0)
        # hoist the ACT exp-table load off the critical path, before the
        # scalar engine's DMA issues
        act_warm = singles.tile([1, 2], BF16, tag="act_warm")
        nc.scalar.activation(act_warm, ones_row[0:1, 0:2], AF.Exp, scale=SCALE)

        # --- bulk input streams ---
        # sync/HWDGE queue: I^T in 6 E-chunks (contiguous 128 rows x 4KB),
        # then x^T in 4 512-col S-blocks (768 rows x 1KB runs).
        # x^T block 0 first so k/v projection work starts (and warms the
        # PE p-state with real work) while I^T streams in behind it.
        IT = singles.tile([128, EC, S], BF16, tag="IT")
        XT = singles.tile([128, EC, S], BF16, tag="XT")

        def dma_xt(blk):
            nc.sync.dma_start(
                out=XT[:, :, blk * 512:(blk + 1) * 512],
                in_=dXT.ap()[:, blk],
            )

        # biases ride the otherwise-idle scalar queue; everything bulky
        # stays on sync: x^T block 0 first (k0/v0 warm the PE with real
        # work from ~10us), then the full I^T stream (gates qT, which
        # gates ALL scores), then x^T blocks 1-3.
        nc.scalar.dma_start(out=b_all, in_=dB.ap())
        nc.scalar.dma_start(out=bv_t, in_=dbv.ap())
        for ei in range(EC):
            nc.sync.dma_start(
                out=IT[:, ei, :], in_=dIT.ap()[ei * 128:(ei + 1) * 128, :]
            )
        for blk in range(NB):
            dma_xt(blk)

        # gpsimd/SWDGE queue (concurrent with the above): whole mask^T as
        # uint8 -> bf16 cast-DMAs in 8 duo-chunks.  The DMA engines are
        # write-bus-bound (~360GB/s aggregate) and the expanded mask is
        # 8MB of SBUF writes vs 6.6MB for all inputs, so duos 2..7 are
        # gated on x^T block arrival (tiny gpsimd reads) to keep the mask
        # stream from starving the inputs that unblock the score phase.
        maskT_all = singles.tile([128, SC, S], BF16, tag="maskT")
        for g in range(SC // 2):
            # Real dependency (not just engine order, which the tile
            # scheduler may reorder): copy one input element INTO this
            # duo's destination region, so the cast-DMA write-after-writes
            # the gate and cannot start before that input chunk landed.
            # Keeps the 8MB of mask SBUF writes from starving the input
            # stream on the shared (write-bound) DMA engines.
            if g == 0:
                gate_src = None          # first duo flows immediately
            else:
                # later duos wait for x^T blocks: the mask streams almost
                # entirely UNDER the score phase, not before it
                blk = min(g, NB - 1)
                gate_src = XT[0:1, 0, blk * 512:blk * 512 + 1]
            if gate_src is not None:
                nc.gpsimd.tensor_copy(maskT_all[0:1, 2 * g, 0:1], gate_src)
            nc.gpsimd.dma_start(
                out=maskT_all[:, 2 * g:2 * g + 2, :],
                in_=dmT.ap()[:, 2 * g:2 * g + 2, :],
            )

        qT = singles.tile([64, S], BF16, tag="qT")
        kT = singles.tile([64, S], BF16, tag="kT")
        vA = singles.tile([128, SC, 66], BF16, tag="vA")
        # ones column for the softmax-denominator trick, set once
        nc.vector.memset(vA[:, :, H:H + 1], 1.0)

        sp = ctx.enter_context(tc.tile_pool(name="sp", bufs=9))
        eep = ctx.enter_context(tc.tile_pool(name="eep", bufs=4))
        outp = ctx.enter_context(tc.tile_pool(name="outp", bufs=1))

        def emit_score(ki, psw):
            """w^T-chunk -> exp -> mask multiply; returns the sT tile.

            Both w halves are emitted before both exps (and both exps
            before both multiplies) so each engine sees its two ops
            back-to-back."""
            maskT_sb = maskT_all[:, ki, :]
            sT_sb = sp.tile([128, S], BF16, tag="sT")
            wps = []
            for hh in range(2):
                wp = psw.tile([128, 1024], F32, tag="w")
                for nb in range(2):
                    nc.tensor.matmul(
                        wp[:, nb * 512:(nb + 1) * 512],
                        lhsT=kT[:, ki * 128:(ki + 1) * 128],
                        rhs=qT[:, (hh * 2 + nb) * 512:(hh * 2 + nb + 1) * 512],
                        start=True,
                        stop=True,
                    )
                wps.append(wp)
            e_sbs = []
            for hh in range(2):
                e_sb = eep.tile([128, 1024], BF16, tag="e")
                nc.scalar.activation(e_sb, wps[hh], AF.Exp, scale=SCALE)
                e_sbs.append(e_sb)
            for hh in range(2):
                nc.vector.tensor_tensor(
                    sT_sb[:, hh * 1024:(hh + 1) * 1024],
                    e_sbs[hh],
                    maskT_sb[:, hh * 1024:(hh + 1) * 1024],
                    ALU.mult,
                )
            return sT_sb

        sTs = {}
        psw = ctx.enter_context(tc.tile_pool(name="psw", bufs=2, space="PSUM"))

        # q projection: E-chunk-major, trailing the I^T stream.  Each nb
        # slice of the accumulator is one 2KB bank so accumulation groups
        # stay bank-local.
        with tc.tile_pool(name="psQ", bufs=1, space="PSUM") as psQ:
            psqall = psQ.tile([64, NB, 512], F32, tag="pq")
            for ei in range(EC):
                for nb in range(NB):
                    nc.tensor.matmul(
                        psqall[:, nb, :],
                        lhsT=w_sb["Wq"][:, ei, :],
                        rhs=IT[:, ei, nb * 512:(nb + 1) * 512],
                        start=(ei == 0),
                        stop=(ei == EC - 1),
                    )
            # bias-adds split DVE / ACT so qT's two halves complete in
            # parallel (S(0)'s first w-matmul needs only nb 0-1)
            for nb in (0, 1):
                nc.vector.tensor_scalar(
                    qT[:, nb * 512:(nb + 1) * 512], psqall[:, nb, :],
                    b_sb["bq"], None, ALU.add,
                )
            for nb in (2, 3):
                nc.scalar.activation(
                    qT[:, nb * 512:(nb + 1) * 512], psqall[:, nb, :],
                    AF.Identity, bias=b_sb["bq"],
                )

        with tc.tile_pool(name="psA", bufs=2, space="PSUM") as psA:

            def emit_k_block(blk):
                psk = psA.tile([64, 512], F32, tag="pk")
                for ei in range(EC):
                    nc.tensor.matmul(
                        psk,
                        lhsT=w_sb["Wk"][:, ei, :],
                        rhs=XT[:, ei, blk * 512:(blk + 1) * 512],
                        start=(ei == 0),
                        stop=(ei == EC - 1),
                    )
                nc.vector.tensor_scalar(
                    kT[:, blk * 512:(blk + 1) * 512], psk, b_sb["bk"], None,
                    ALU.add,
                )

            def emit_v_proj(kb):
                psv = psA.tile([128, H], F32, tag="pv")
                for ei in range(EC):
                    nc.tensor.matmul(
                        psv,
                        lhsT=XT[:, ei, kb * 128:(kb + 1) * 128],
                        rhs=w_sb["Wv"][:, ei, :],
                        start=(ei == 0),
                        stop=False,
                    )
                nc.tensor.matmul(
                    psv,
                    lhsT=ones_row[:, 0:128],
                    rhs=b_sb["bv"],
                    start=False,
                    stop=True,
                )
                nc.vector.tensor_copy(vA[:, kb, 0:H], psv)

            # k/v for block b+1 always emitted BEFORE scores for block b:
            # the in-order PE must never queue backpressured score matmuls
            # ahead of projection work whose inputs are already resident.
            # Remaining v-projections are fine-grain-interleaved between
            # score chunks: a 0.5us v-proj fits the PE slack per exp,
            # while a 4-at-a-time lump (~3us) would drain ACT's one-chunk
            # buffer.
            emit_k_block(0)
            for kb in range(4):
                emit_v_proj(kb)
            emit_k_block(1)
            for kb in range(4, 8):
                emit_v_proj(kb)
            for ki in range(0, 4):
                sTs[ki] = emit_score(ki, psw)
                emit_v_proj(8 + ki)
            emit_k_block(2)
            for ki in range(4, 8):
                sTs[ki] = emit_score(ki, psw)
                emit_v_proj(12 + ki - 4)
            emit_k_block(3)

        # ---- ctx accumulation (psA's + psw1's banks freed above) ----
        psctx = ctx.enter_context(
            tc.tile_pool(name="psctx", bufs=1, space="PSUM")
        )

        # [q_within, qj, 64 ctx + 1 denom + pad] — 128-wide regions keep
        # each accumulation group inside one PSUM bank.
        ctxall = psctx.tile([128, SC, 128], F32, tag="ctxall")

        def emit_ctx(ki):
            # start=True zeroes the whole 2KB PSUM bank, so only the first
            # matmul touching each bank (4 qj regions per bank) gets it;
            # stop on the bank's last matmul.
            sT_sb = sTs.pop(ki)
            for qj in range(SC):
                nc.tensor.matmul(
                    ctxall[:, qj, 0:H + 1],
                    lhsT=sT_sb[:, qj * 128:(qj + 1) * 128],
                    rhs=vA[:, ki, 0:H + 1],
                    start=(ki == 0 and qj % 4 == 0),
                    stop=(ki == SC - 1 and qj % 4 == 3),
                )

        for ki in range(8, SC):
            sTs[ki] = emit_score(ki, psw)
            emit_ctx(ki - 8)
        for ki in range(8, SC):
            emit_ctx(ki)

        # epilogue split by PSUM bank (4 qj per bank): each bank's
        # reciprocal/multiply/output-DMA starts as soon as its last ctx
        # accumulation closes, instead of waiting for the whole ctx tile
        recip_t = outp.tile([128, SC, 1], F32, tag="recip")
        o_all = outp.tile([128, SC, H], F32, tag="o")
        dview = dout.ap().rearrange("(p qj) h -> p qj h", p=128)
        for bank in range(4):
            qlo, qhi = 4 * bank, 4 * bank + 4
            nc.vector.reciprocal(
                recip_t[:, qlo:qhi, :], ctxall[:, qlo:qhi, H:H + 1]
            )
            rsl = recip_t[:, qlo:qhi, :]
            recip_bcast = bass.AP(
                tensor=rsl.tensor,
                offset=rsl.offset,
                ap=[rsl.ap[0], rsl.ap[1], [0, H]],
            )
            nc.vector.tensor_tensor(
                o_all[:, qlo:qhi, :], ctxall[:, qlo:qhi, 0:H], recip_bcast,
                ALU.mult,
            )
            nc.sync.dma_start(
                out=dview[:, qlo:qhi, :], in_=o_all[:, qlo:qhi, :]
            )

    nc.compile()
    return nc


def get_program():
    if "nc" not in _cache:
        _cache["nc"] = _build_program()
    return _cache["nc"]


def make_in_maps(I, x, mask, Wq, bq, Wk, bk, Wv, bv):
    import ml_dtypes

    BF = ml_dtypes.bfloat16
    FP8NP = ml_dtypes.float8_e4m3
    I = np.asarray(I, dtype=np.float32)
    x = np.asarray(x, dtype=np.float32)
    mask = np.asarray(mask, dtype=np.int32)

    Wpack = np.concatenate(
        [
            np.asarray(Wq, dtype=np.float32).astype(BF),
            np.asarray(Wk, dtype=np.float32).astype(BF),
            np.asarray(Wv, dtype=np.float32).astype(BF),
        ],
        axis=1,
    )
    # partition-major: Wp[p, ec, h] = Wpack[ec*128 + p, h]
    Wp = np.ascontiguousarray(
        Wpack.reshape(EC, 128, 3 * H).transpose(1, 0, 2)
    )
    bpack = np.concatenate(
        [
            np.broadcast_to(np.asarray(bq, np.float32).reshape(H, 1), (H, H)),
            np.broadcast_to(np.asarray(bk, np.float32).reshape(H, 1), (H, H)),
        ],
        axis=1,
    ).astype(np.float32)
    bv = np.asarray(bv, dtype=np.float32).reshape(1, H).astype(BF)

    in_maps = []
    for b in range(B):
        xT = x[b].T.astype(BF)                     # [E, S]
        # XTp[p, blk, ec, c] = xT[ec*128 + p, blk*512 + c]
        XTp = np.ascontiguousarray(
            xT.reshape(EC, 128, NB, 512).transpose(1, 2, 0, 3)
        )
        mT = mask[b].T.astype(np.uint8)            # [S(k), S(q)]
        # maskTp[p, ki, q] = mT[ki*128 + p, q]
        mTp = np.ascontiguousarray(
            mT.reshape(SC, 128, S).transpose(1, 0, 2)
        )
        in_maps.append(
            {
                "IT": np.ascontiguousarray(I[b].T).astype(BF),
                "XTp": XTp,
                "maskTp": mTp,
                "Wpack": Wp, "bpack": bpack, "bv": bv,
            }
        )
    return in_maps


def unpermute_out(dev_out):
    # device wrote row (p*16 + qj) = logical q row (qj*128 + p)
    return (
        np.asarray(dev_out).reshape(128, SC, H).transpose(1, 0, 2).reshape(S, H)
    )


def kernel(I, x, mask, Wq, bq, Wk, bk, Wv, bv):
    nc = get_program()
    in_maps = make_in_maps(I, x, mask, Wq, bq, Wk, bk, Wv, bv)
    res = run_bass_kernel_spmd(nc, in_maps, list(range(N_CORES)))
    out = np.stack(
        [unpermute_out(res.results[b]["out"]) for b in range(B)], axis=0
    )
    return out.astype(np.float32)
